# revision 12
# baseline (speedup 1.0000x reference)
# Trainium2 Bass kernel for ConvSelfAttn3D:
#   out = scale * (softmax(g @ f^T) @ h) @ Wv + x   (single head, N=4096, d=32)
#
# Sharding: 8 cores = 4 batches x 2 query-halves. Each core computes the
# full attention for its 2048 queries against all 4096 keys of its batch.
#
# Per-core layout strategy ("S-transposed flash"):
#   - All matmul operands kept in bf16 (4x faster PE than fp32), fp32 PSUM.
#   - Scores are computed transposed: S^T[key, q] via
#       matmul(lhsT=f^T tile [32,128], rhs=g^T [32, 512])
#     so softmax exp runs on [128 keys, q] tiles and the PV matmul
#       matmul(lhsT=h_aug [128,33], rhs=expS^T [128,512])
#     needs no transposes at all. h_aug has a ones column, so PV's
#     partition 32 accumulates sum_k exp(s) = the softmax denominator.
#   - Softmax max-subtraction is skipped: scores are ~N(0,1), |s| < ~6.
#   - Division by the denominator + residual happen at the very end in
#     natural layout after one PE transpose of [65, 128] tiles.
#
# Host-side prep is layout-only: transposes/casts of inputs, bias folding
# (ones rows so biases ride along in the matmuls), scale folded into Wv.

import numpy as np
import ml_dtypes

B, N, C = 4, 4096, 64
D = 32          # attn dim
NQ = N // 2     # queries per core
KT = N // 128   # 32 key tiles
NCORES = 8

_BF16 = ml_dtypes.bfloat16

# Fraction of exp tiles handled by VectorE (Schraudolph bf16 exp approx)
# instead of ScalarE ACT exp: iteration i goes to VectorE if (i % DEN) < NUM.
EXP_VEC_NUM, EXP_VEC_DEN = 1, 2
SCHRAU_A = 128.0 / float(np.log(2.0))
SCHRAU_B = 16250.5
_cache = {}


def _f32(a):
    return np.ascontiguousarray(a, dtype=np.float32)


def _bf(a):
    return np.ascontiguousarray(np.asarray(a, dtype=np.float32).astype(_BF16))


def make_shards(x, Wf, bf, Wg, bg, Wh, bh, Wv, bv, scale):
    """Host-side, layout-only sharding of the full inputs into 8 per-core maps."""
    x2 = _f32(x).reshape(B, N, C)
    ones = np.ones((1, N), np.float32)

    wf = _bf(np.concatenate([_f32(Wf), _f32(bf).reshape(1, D)], 0))        # [65,32]
    wg1 = np.concatenate([_f32(Wg), _f32(bg).reshape(1, D)], 0)
    wg = _bf(np.concatenate([wg1, wg1], 1))                                # [65,64]
    wh_aug = np.zeros((C + 1, D + 1), np.float32)
    wh_aug[:C, :D] = _f32(Wh)
    wh_aug[C, :D] = _f32(bh)
    wh_aug[C, D] = 1.0                                                     # ones col
    wh = _bf(wh_aug)                                                       # [65,33]
    wv = _bf(float(scale) * _f32(Wv))                                      # [32,64]
    bvs = _f32(float(scale) * _f32(bv)).reshape(C, 1)                      # [64,1]
    ident = _bf(np.eye(128, dtype=np.float32))

    in_maps = []
    for core in range(NCORES):
        b, qh = core // 2, core % 2
        xT = np.concatenate([x2[b].T, ones], 0)                            # [65,4096]
        q0 = qh * NQ
        in_maps.append({
            "xT": _bf(xT),
            "xTq": _bf(xT[:, q0:q0 + NQ]),
            "xq": _f32(x2[b, q0:q0 + NQ]),
            "wf": wf, "wg": wg, "wh": wh, "wv": wv, "bv": bvs,
            "ident": ident,
        })
    return in_maps


def build_nc():
    import concourse.mybir as mybir
    import concourse.tile as tile
    from concourse import bacc

    f32 = mybir.dt.float32
    bf16 = mybir.dt.bfloat16
    EXP = mybir.ActivationFunctionType.Exp
    MUL = mybir.AluOpType.mult
    ADD = mybir.AluOpType.add

    nc = bacc.Bacc("TRN2", target_bir_lowering=False, debug=False,
                   enable_asserts=False)

    dram = {}
    for name, shape, dt in [
        ("xT", [C + 1, N], bf16), ("xTq", [C + 1, NQ], bf16),
        ("xq", [NQ, C], f32),
        ("wf", [C + 1, D], bf16), ("wg", [C + 1, 2 * D], bf16),
        ("wh", [C + 1, D + 1], bf16), ("wv", [D, C], bf16),
        ("bv", [C, 1], f32), ("ident", [128, 128], bf16),
    ]:
        dram[name] = nc.dram_tensor(name, shape, dt, kind="ExternalInput").ap()
    out_d = nc.dram_tensor("out", [NQ, C], f32, kind="ExternalOutput").ap()

    from contextlib import ExitStack

    with tile.TileContext(nc) as tc, ExitStack() as ctx:
        consts = ctx.enter_context(tc.tile_pool(name="consts", bufs=1))
        big = ctx.enter_context(tc.tile_pool(name="big", bufs=1))
        ps = ctx.enter_context(tc.tile_pool(name="ps", bufs=4, space="PSUM"))
        cps = ctx.enter_context(tc.tile_pool(name="cps", bufs=1, space="PSUM"))
        xs = ctx.enter_context(tc.tile_pool(name="xs", bufs=4))
        small = ctx.enter_context(tc.tile_pool(name="small", bufs=4))
        outp = ctx.enter_context(tc.tile_pool(name="outp", bufs=4))

        # ---- load constants / inputs
        wf_sb = consts.tile([C + 1, D], bf16)
        wg_sb = consts.tile([C + 1, 2 * D], bf16)
        wh_sb = consts.tile([C + 1, D + 1], bf16)
        wv_sb = consts.tile([D, C], bf16)
        bv_sb = consts.tile([C, 1], f32)
        id_sb = consts.tile([128, 128], bf16)
        for t, name in [(wf_sb, "wf"), (wg_sb, "wg"), (wh_sb, "wh"),
                        (wv_sb, "wv"), (bv_sb, "bv"), (id_sb, "ident")]:
            nc.sync.dma_start(out=t, in_=dram[name])

        xT_sb = big.tile([C + 1, N], bf16)
        xTq_sb = big.tile([C + 1, NQ], bf16)
        xq_sb = big.tile([128, (NQ // 128) * C], f32)
        for c in range(4):
            sl = slice(c * 1024, (c + 1) * 1024)
            nc.sync.dma_start(out=xT_sb[:, sl], in_=dram["xT"][:, sl])
        nc.gpsimd.dma_start(out=xTq_sb, in_=dram["xTq"])
        nc.gpsimd.dma_start(
            out=xq_sb[:].rearrange("p (t c) -> p t c", c=C),
            in_=dram["xq"].rearrange("(t p) c -> p t c", p=128),
        )

        # ---- projections.
        # fT_pk [64, 2048]: partitions 32i+d hold f^T[d, keys of tile 2p+i]
        # at free p*128+j (kt pairs packed for 2x row-tiled QK).
        # gT_rep [64, 2048]: g^T replicated on partitions 0-31 / 32-63.
        # h_sb [128, 32*33]: h_aug natural per key tile (col 32 = ones).
        fT_pk = big.tile([2 * D, NQ], bf16)
        gT_rep = big.tile([2 * D, NQ], bf16)
        h_sb = big.tile([128, KT * (D + 1)], bf16)

        xT_r = xT_sb[:].rearrange("k (p i j) -> k i p j", i=2, j=128)
        for c in range(4):
            pt = ps.tile([2 * D, 512], f32, tag="ps")
            for i in range(2):
                nc.tensor.matmul(pt[32 * i:32 * (i + 1), :], wf_sb,
                                 xT_r[:, i, 4 * c:4 * (c + 1), :])
            if c % 2 == 0:
                nc.scalar.copy(out=fT_pk[:, c * 512:(c + 1) * 512], in_=pt)
            else:
                nc.vector.tensor_copy(out=fT_pk[:, c * 512:(c + 1) * 512], in_=pt)

        for c in range(4):
            pt = ps.tile([2 * D, 512], f32, tag="ps")
            nc.tensor.matmul(pt, wg_sb, xTq_sb[:, c * 512:(c + 1) * 512])
            if c % 2 == 0:
                nc.scalar.copy(out=gT_rep[:, c * 512:(c + 1) * 512], in_=pt)
            else:
                nc.vector.tensor_copy(out=gT_rep[:, c * 512:(c + 1) * 512], in_=pt)

        W1 = D + 1
        for g8 in range(KT // 4):
            pt = ps.tile([128, 4 * W1], f32, tag="ps")
            for i in range(4):
                kt = g8 * 4 + i
                nc.tensor.matmul(pt[:, i * W1:(i + 1) * W1],
                                 xT_sb[:, kt * 128:(kt + 1) * 128], wh_sb)
            if g8 % 2 == 0:
                nc.scalar.copy(out=h_sb[:, g8 * 4 * W1:(g8 + 1) * 4 * W1], in_=pt)
            else:
                nc.vector.tensor_copy(out=h_sb[:, g8 * 4 * W1:(g8 + 1) * 4 * W1], in_=pt)

        # ---- main flash loop over key-tile pairs x 512-query chunks.
        # QK: 2x row-tiled (K=32 at partitions 0/32); even kt -> s[:,0:512],
        # odd kt -> s[:,512:1024]. PV: 2x col-tiled; even kt accumulates at
        # ctx partitions 0:33, odd kt at 64:97 (merged in the epilogue).
        ctx = cps.tile([97, NQ], f32)             # 4 banks
        for p in range(KT // 2):
            for qc in range(NQ // 512):
                q0 = qc * 512
                sA = ps.tile([128, 512], f32, tag="ps")
                sB = ps.tile([128, 512], f32, tag="ps")
                with tc.tile_critical():
                    nc.tensor.matmul(sA, fT_pk[0:D, p * 128:(p + 1) * 128],
                                     gT_rep[0:D, q0:q0 + 512])
                    nc.tensor.matmul(sB, fT_pk[D:2 * D, p * 128:(p + 1) * 128],
                                     gT_rep[D:2 * D, q0:q0 + 512])
                exA = xs.tile([128, 512], bf16, tag="ex")
                exB = xs.tile([128, 512], bf16, tag="ex")
                nc.scalar.activation(out=exA, in_=sA, func=EXP)
                # Schraudolph bf16 exp: i16 = round(s*128/ln2 + B)
                nc.vector.tensor_scalar(
                    out=exB[:].bitcast(mybir.dt.int16), in0=sB,
                    scalar1=SCHRAU_A, scalar2=SCHRAU_B, op0=MUL, op1=ADD)
                with tc.tile_critical():
                    nc.tensor.matmul(ctx[0:D + 1, q0:q0 + 512],
                                     h_sb[:, 2 * p * W1:(2 * p + 1) * W1],
                                     exA,
                                     start=(p == 0), stop=(p == KT // 2 - 1),
                                     skip_group_check=True)
                    nc.tensor.matmul(ctx[64:64 + D + 1, q0:q0 + 512],
                                     h_sb[:, (2 * p + 1) * W1:(2 * p + 2) * W1],
                                     exB,
                                     start=(p == 0), stop=(p == KT // 2 - 1),
                                     skip_group_check=True)

        # ---- epilogue: merge even/odd ctx halves, v^T = Wv_s^T @ ctx^T
        #      (+bv), append sumexp row, transpose to natural, divide,
        #      add residual, store.
        ctxA_sb = big.tile([D + 1, NQ], bf16)
        ctxT_sb = big.tile([D + 1, NQ], bf16)
        for hh in range(2):
            sl = slice(hh * 1024, (hh + 1) * 1024)
            nc.scalar.copy(out=ctxA_sb[:, sl], in_=ctx[0:D + 1, sl])
            nc.vector.tensor_tensor(out=ctxT_sb[:, sl], in0=ctxA_sb[:, sl],
                                    in1=ctx[64:64 + D + 1, sl], op=ADD)

        vT_sb = big.tile([C + 1, NQ], bf16)
        for qc in range(NQ // 512):
            vt = ps.tile([C, 512], f32, tag="ps")
            nc.tensor.matmul(vt, wv_sb, ctxT_sb[0:D, qc * 512:(qc + 1) * 512])
            if qc % 2 == 0:
                nc.vector.tensor_scalar(
                    out=vT_sb[0:C, qc * 512:(qc + 1) * 512], in0=vt,
                    scalar1=bv_sb, scalar2=None, op0=ADD)
            else:
                nc.scalar.activation(
                    out=vT_sb[0:C, qc * 512:(qc + 1) * 512], in_=vt,
                    func=mybir.ActivationFunctionType.Identity, bias=bv_sb)
        # sumexp row rides along as partition 64
        nc.vector.tensor_copy(out=vT_sb[C:C + 1, :], in_=ctxT_sb[D:D + 1, :])

        for qt in range(NQ // 128):
            tp = ps.tile([128, C + 1], bf16, tag="ps")
            nc.tensor.transpose(tp, vT_sb[:, qt * 128:(qt + 1) * 128],
                                id_sb[0:C + 1, 0:C + 1])
            r = small.tile([128, 1], f32, tag="r")
            nc.vector.reciprocal(r, tp[:, C:C + 1])
            tmp = outp.tile([128, C], f32, tag="tmp")
            nc.scalar.activation(out=tmp, in_=tp[:, 0:C],
                                 func=mybir.ActivationFunctionType.Copy, scale=r)
            ot = outp.tile([128, C], f32, tag="ot")
            nc.vector.tensor_tensor(out=ot, in0=tmp,
                                    in1=xq_sb[:, qt * C:(qt + 1) * C], op=ADD)
            nc.sync.dma_start(out=out_d[qt * 128:(qt + 1) * 128, :], in_=ot)

    nc.compile()
    return nc


def get_nc():
    if "nc" not in _cache:
        _cache["nc"] = build_nc()
    return _cache["nc"]


def kernel(**inputs):
    from concourse.bass_utils import run_bass_kernel_spmd

    nc = get_nc()
    in_maps = make_shards(**inputs)
    res = run_bass_kernel_spmd(nc, in_maps, core_ids=list(range(NCORES)))
    out = np.empty((B, N, C), np.float32)
    for core in range(NCORES):
        b, qh = core // 2, core % 2
        out[b, qh * NQ:(qh + 1) * NQ] = res.results[core]["out"]
    return out.reshape(B, 16, 16, 16, C)


# revision 14
# speedup vs baseline: 2.0266x; 2.0266x over previous
# Trainium2 Bass kernel for ConvSelfAttn3D:
#   out = scale * (softmax(g @ f^T) @ h) @ Wv + x   (single head, N=4096, d=32)
#
# Sharding: 8 cores = 4 batches x 2 query-halves. Each core computes the
# full attention for its 2048 queries against all 4096 keys of its batch.
#
# Per-core layout strategy ("S-transposed flash"):
#   - All matmul operands kept in bf16 (4x faster PE than fp32), fp32 PSUM.
#   - Scores are computed transposed: S^T[key, q] via
#       matmul(lhsT=f^T tile [32,128], rhs=g^T [32, 512])
#     so softmax exp runs on [128 keys, q] tiles and the PV matmul
#       matmul(lhsT=h_aug [128,33], rhs=expS^T [128,512])
#     needs no transposes at all. h_aug has a ones column, so PV's
#     partition 32 accumulates sum_k exp(s) = the softmax denominator.
#   - Softmax max-subtraction is skipped: scores are ~N(0,1), |s| < ~6.
#   - Division by the denominator + residual happen at the very end in
#     natural layout after one PE transpose of [65, 128] tiles.
#
# Host-side prep is layout-only: transposes/casts of inputs, bias folding
# (ones rows so biases ride along in the matmuls), scale folded into Wv.

import numpy as np
import ml_dtypes

B, N, C = 4, 4096, 64
D = 32          # attn dim
NQ = N // 2     # queries per core
KT = N // 128   # 32 key tiles
NCORES = 8

_BF16 = ml_dtypes.bfloat16

# Fraction of exp tiles handled by VectorE (Schraudolph bf16 exp approx)
# instead of ScalarE ACT exp: iteration i goes to VectorE if (i % DEN) < NUM.
EXP_VEC_NUM, EXP_VEC_DEN = 1, 2
SCHRAU_A = 128.0 / float(np.log(2.0))
SCHRAU_B = 16250.5
_cache = {}


def _f32(a):
    return np.ascontiguousarray(a, dtype=np.float32)


def _bf(a):
    return np.ascontiguousarray(np.asarray(a, dtype=np.float32).astype(_BF16))


def make_shards(x, Wf, bf, Wg, bg, Wh, bh, Wv, bv, scale):
    """Host-side, layout-only sharding of the full inputs into 8 per-core maps."""
    x2 = _f32(x).reshape(B, N, C)
    ones = np.ones((1, N), np.float32)

    wf = _bf(np.concatenate([_f32(Wf), _f32(bf).reshape(1, D)], 0))        # [65,32]
    wg1 = np.concatenate([_f32(Wg), _f32(bg).reshape(1, D)], 0)
    wg = _bf(np.concatenate([wg1, wg1], 1))                                # [65,64]
    wh_aug = np.zeros((C + 1, D + 1), np.float32)
    wh_aug[:C, :D] = _f32(Wh)
    wh_aug[C, :D] = _f32(bh)
    wh_aug[C, D] = 1.0                                                     # ones col
    wh = _bf(wh_aug)                                                       # [65,33]
    wv = _bf(float(scale) * _f32(Wv))                                      # [32,64]
    bvs = _f32(float(scale) * _f32(bv)).reshape(C, 1)                      # [64,1]
    ident = _bf(np.eye(128, dtype=np.float32))

    in_maps = []
    for core in range(NCORES):
        b, qh = core // 2, core % 2
        xT = np.concatenate([x2[b].T, ones], 0)                            # [65,4096]
        q0 = qh * NQ
        in_maps.append({
            "xT": _bf(xT),
            "xTq": _bf(xT[:, q0:q0 + NQ]),
            "xq": _f32(x2[b, q0:q0 + NQ]),
            "wf": wf, "wg": wg, "wh": wh, "wv": wv, "bv": bvs,
            "ident": ident,
        })
    return in_maps


def build_nc():
    import concourse.mybir as mybir
    import concourse.tile as tile
    from concourse import bacc

    f32 = mybir.dt.float32
    bf16 = mybir.dt.bfloat16
    EXP = mybir.ActivationFunctionType.Exp
    MUL = mybir.AluOpType.mult
    ADD = mybir.AluOpType.add

    nc = bacc.Bacc("TRN2", target_bir_lowering=False, debug=False,
                   enable_asserts=False)

    dram = {}
    for name, shape, dt in [
        ("xT", [C + 1, N], bf16), ("xTq", [C + 1, NQ], bf16),
        ("xq", [NQ, C], f32),
        ("wf", [C + 1, D], bf16), ("wg", [C + 1, 2 * D], bf16),
        ("wh", [C + 1, D + 1], bf16), ("wv", [D, C], bf16),
        ("bv", [C, 1], f32), ("ident", [128, 128], bf16),
    ]:
        dram[name] = nc.dram_tensor(name, shape, dt, kind="ExternalInput").ap()
    out_d = nc.dram_tensor("out", [NQ, C], f32, kind="ExternalOutput").ap()

    from contextlib import ExitStack

    with tile.TileContext(nc) as tc, ExitStack() as ctx:
        consts = ctx.enter_context(tc.tile_pool(name="consts", bufs=1))
        big = ctx.enter_context(tc.tile_pool(name="big", bufs=1))
        ps = ctx.enter_context(tc.tile_pool(name="ps", bufs=4, space="PSUM"))
        cps = ctx.enter_context(tc.tile_pool(name="cps", bufs=1, space="PSUM"))
        xs = ctx.enter_context(tc.tile_pool(name="xs", bufs=4))
        small = ctx.enter_context(tc.tile_pool(name="small", bufs=4))
        outp = ctx.enter_context(tc.tile_pool(name="outp", bufs=4))

        # ---- load constants / inputs
        wf_sb = consts.tile([C + 1, D], bf16)
        wg_sb = consts.tile([C + 1, 2 * D], bf16)
        wh_sb = consts.tile([C + 1, D + 1], bf16)
        wv_sb = consts.tile([D, C], bf16)
        bv_sb = consts.tile([C, 1], f32)
        id_sb = consts.tile([128, 128], bf16)
        for t, name in [(wf_sb, "wf"), (wg_sb, "wg"), (wh_sb, "wh"),
                        (wv_sb, "wv"), (bv_sb, "bv"), (id_sb, "ident")]:
            nc.sync.dma_start(out=t, in_=dram[name])

        xT_sb = big.tile([C + 1, N], bf16)
        xTq_sb = big.tile([C + 1, NQ], bf16)
        xq_sb = big.tile([128, (NQ // 128) * C], f32)
        for c in range(4):
            sl = slice(c * 1024, (c + 1) * 1024)
            nc.sync.dma_start(out=xT_sb[:, sl], in_=dram["xT"][:, sl])
        nc.gpsimd.dma_start(out=xTq_sb, in_=dram["xTq"])
        nc.gpsimd.dma_start(
            out=xq_sb[:].rearrange("p (t c) -> p t c", c=C),
            in_=dram["xq"].rearrange("(t p) c -> p t c", p=128),
        )

        # ---- projections.
        # fT_pk [64, 2048]: partitions 32i+d hold f^T[d, keys of tile 2p+i]
        # at free p*128+j (kt pairs packed for 2x row-tiled QK).
        # gT_rep [64, 2048]: g^T replicated on partitions 0-31 / 32-63.
        # h_sb [128, 32*33]: h_aug natural per key tile (col 32 = ones).
        fT_pk = big.tile([2 * D, NQ], bf16)
        gT_rep = big.tile([2 * D, NQ], bf16)
        h_sb = big.tile([128, KT * (D + 1)], bf16)

        xT_r = xT_sb[:].rearrange("k (p i j) -> k i p j", i=2, j=128)
        for c in range(4):
            pt = ps.tile([2 * D, 512], f32, tag="ps")
            for i in range(2):
                nc.tensor.matmul(pt[32 * i:32 * (i + 1), :], wf_sb,
                                 xT_r[:, i, 4 * c:4 * (c + 1), :])
            if c % 2 == 0:
                nc.scalar.copy(out=fT_pk[:, c * 512:(c + 1) * 512], in_=pt)
            else:
                nc.vector.tensor_copy(out=fT_pk[:, c * 512:(c + 1) * 512], in_=pt)

        for c in range(4):
            pt = ps.tile([2 * D, 512], f32, tag="ps")
            nc.tensor.matmul(pt, wg_sb, xTq_sb[:, c * 512:(c + 1) * 512])
            if c % 2 == 0:
                nc.scalar.copy(out=gT_rep[:, c * 512:(c + 1) * 512], in_=pt)
            else:
                nc.vector.tensor_copy(out=gT_rep[:, c * 512:(c + 1) * 512], in_=pt)

        W1 = D + 1
        for g8 in range(KT // 4):
            pt = ps.tile([128, 4 * W1], f32, tag="ps")
            for i in range(4):
                kt = g8 * 4 + i
                nc.tensor.matmul(pt[:, i * W1:(i + 1) * W1],
                                 xT_sb[:, kt * 128:(kt + 1) * 128], wh_sb)
            if g8 % 2 == 0:
                nc.scalar.copy(out=h_sb[:, g8 * 4 * W1:(g8 + 1) * 4 * W1], in_=pt)
            else:
                nc.vector.tensor_copy(out=h_sb[:, g8 * 4 * W1:(g8 + 1) * 4 * W1], in_=pt)

        # ---- main flash loop over key-tile pairs x 512-query chunks.
        # QK: 2x row-tiled (K=32 at partitions 0/32); even kt -> s[:,0:512],
        # odd kt -> s[:,512:1024]. PV: 2x col-tiled; even kt accumulates at
        # ctx partitions 0:33, odd kt at 64:97 (merged in the epilogue).
        ctx = cps.tile([97, NQ], f32)             # 4 banks
        prev_pv = None
        for p in range(KT // 2):
            for qc in range(NQ // 512):
                q0 = qc * 512
                sA = ps.tile([128, 512], f32, tag="ps")
                sB = ps.tile([128, 512], f32, tag="ps")
                qk_a = nc.tensor.matmul(sA, fT_pk[0:D, p * 128:(p + 1) * 128],
                                        gT_rep[0:D, q0:q0 + 512])
                nc.tensor.matmul(sB, fT_pk[D:2 * D, p * 128:(p + 1) * 128],
                                 gT_rep[D:2 * D, q0:q0 + 512])
                if prev_pv is not None:
                    # Keep PE queue order [QK pair][PV pair] so tile_position
                    # pairs stay adjacent and run concurrently.
                    tile.add_dep_helper(qk_a.ins, prev_pv.ins, sync=False,
                                        reason="pair adjacency")
                exA = xs.tile([128, 512], bf16, tag="ex")
                exB = xs.tile([128, 512], bf16, tag="ex")
                nc.scalar.activation(out=exA, in_=sA, func=EXP)
                # Schraudolph bf16 exp: i16 = round(s*128/ln2 + B)
                nc.vector.tensor_scalar(
                    out=exB[:].bitcast(mybir.dt.int16), in0=sB,
                    scalar1=SCHRAU_A, scalar2=SCHRAU_B, op0=MUL, op1=ADD)
                nc.tensor.matmul(ctx[0:D + 1, q0:q0 + 512],
                                 h_sb[:, 2 * p * W1:(2 * p + 1) * W1],
                                 exA,
                                 start=(p == 0), stop=(p == KT // 2 - 1),
                                 skip_group_check=True)
                prev_pv = nc.tensor.matmul(
                    ctx[64:64 + D + 1, q0:q0 + 512],
                    h_sb[:, (2 * p + 1) * W1:(2 * p + 2) * W1],
                    exB,
                    start=(p == 0), stop=(p == KT // 2 - 1),
                    skip_group_check=True)

        # ---- epilogue: merge even/odd ctx halves, v^T = Wv_s^T @ ctx^T
        #      (+bv), append sumexp row, transpose to natural, divide,
        #      add residual, store.
        ctxA_sb = big.tile([D + 1, NQ], bf16)
        ctxT_sb = big.tile([D + 1, NQ], bf16)
        for hh in range(2):
            sl = slice(hh * 1024, (hh + 1) * 1024)
            nc.scalar.copy(out=ctxA_sb[:, sl], in_=ctx[0:D + 1, sl])
            nc.vector.tensor_tensor(out=ctxT_sb[:, sl], in0=ctxA_sb[:, sl],
                                    in1=ctx[64:64 + D + 1, sl], op=ADD)

        vT_sb = big.tile([C + 1, NQ], bf16)
        for qc in range(NQ // 512):
            vt = ps.tile([C, 512], f32, tag="ps")
            nc.tensor.matmul(vt, wv_sb, ctxT_sb[0:D, qc * 512:(qc + 1) * 512])
            if qc % 2 == 0:
                nc.vector.tensor_scalar(
                    out=vT_sb[0:C, qc * 512:(qc + 1) * 512], in0=vt,
                    scalar1=bv_sb, scalar2=None, op0=ADD)
            else:
                nc.scalar.activation(
                    out=vT_sb[0:C, qc * 512:(qc + 1) * 512], in_=vt,
                    func=mybir.ActivationFunctionType.Identity, bias=bv_sb)
        # sumexp row rides along as partition 64
        nc.vector.tensor_copy(out=vT_sb[C:C + 1, :], in_=ctxT_sb[D:D + 1, :])

        for qt in range(NQ // 128):
            tp = ps.tile([128, C + 1], bf16, tag="ps")
            nc.tensor.transpose(tp, vT_sb[:, qt * 128:(qt + 1) * 128],
                                id_sb[0:C + 1, 0:C + 1])
            r = small.tile([128, 1], f32, tag="r")
            nc.vector.reciprocal(r, tp[:, C:C + 1])
            tmp = outp.tile([128, C], f32, tag="tmp")
            nc.scalar.activation(out=tmp, in_=tp[:, 0:C],
                                 func=mybir.ActivationFunctionType.Copy, scale=r)
            ot = outp.tile([128, C], f32, tag="ot")
            nc.vector.tensor_tensor(out=ot, in0=tmp,
                                    in1=xq_sb[:, qt * C:(qt + 1) * C], op=ADD)
            nc.sync.dma_start(out=out_d[qt * 128:(qt + 1) * 128, :], in_=ot)

    nc.compile()
    return nc


def get_nc():
    if "nc" not in _cache:
        _cache["nc"] = build_nc()
    return _cache["nc"]


def kernel(**inputs):
    from concourse.bass_utils import run_bass_kernel_spmd

    nc = get_nc()
    in_maps = make_shards(**inputs)
    res = run_bass_kernel_spmd(nc, in_maps, core_ids=list(range(NCORES)))
    out = np.empty((B, N, C), np.float32)
    for core in range(NCORES):
        b, qh = core // 2, core % 2
        out[b, qh * NQ:(qh + 1) * NQ] = res.results[core]["out"]
    return out.reshape(B, 16, 16, 16, C)


# revision 15
# speedup vs baseline: 2.8639x; 1.4132x over previous
# Trainium2 Bass kernel for ConvSelfAttn3D:
#   out = scale * (softmax(g @ f^T) @ h) @ Wv + x   (single head, N=4096, d=32)
#
# Sharding: 8 cores = 4 batches x 2 query-halves. Each core computes the
# full attention for its 2048 queries against all 4096 keys of its batch.
#
# Per-core layout strategy ("S-transposed flash"):
#   - All matmul operands kept in bf16 (4x faster PE than fp32), fp32 PSUM.
#   - Scores are computed transposed: S^T[key, q] via
#       matmul(lhsT=f^T tile [32,128], rhs=g^T [32, 512])
#     so softmax exp runs on [128 keys, q] tiles and the PV matmul
#       matmul(lhsT=h_aug [128,33], rhs=expS^T [128,512])
#     needs no transposes at all. h_aug has a ones column, so PV's
#     partition 32 accumulates sum_k exp(s) = the softmax denominator.
#   - Softmax max-subtraction is skipped: scores are ~N(0,1), |s| < ~6.
#   - Division by the denominator + residual happen at the very end in
#     natural layout after one PE transpose of [65, 128] tiles.
#
# Host-side prep is layout-only: transposes/casts of inputs, bias folding
# (ones rows so biases ride along in the matmuls), scale folded into Wv.

import numpy as np
import ml_dtypes

B, N, C = 4, 4096, 64
D = 32          # attn dim
NQ = N // 2     # queries per core
KT = N // 128   # 32 key tiles
NCORES = 8

_BF16 = ml_dtypes.bfloat16

# Fraction of exp tiles handled by VectorE (Schraudolph bf16 exp approx)
# instead of ScalarE ACT exp: iteration i goes to VectorE if (i % DEN) < NUM.
EXP_VEC_NUM, EXP_VEC_DEN = 1, 2
SCHRAU_A = 128.0 / float(np.log(2.0))
SCHRAU_B = 16250.5
_cache = {}


def _f32(a):
    return np.ascontiguousarray(a, dtype=np.float32)


def _bf(a):
    return np.ascontiguousarray(np.asarray(a, dtype=np.float32).astype(_BF16))


def make_shards(x, Wf, bf, Wg, bg, Wh, bh, Wv, bv, scale):
    """Host-side, layout-only sharding of the full inputs into 8 per-core maps."""
    x2 = _f32(x).reshape(B, N, C)
    ones = np.ones((1, N), np.float32)

    wf = _bf(np.concatenate([_f32(Wf), _f32(bf).reshape(1, D)], 0))        # [65,32]
    wg1 = np.concatenate([_f32(Wg), _f32(bg).reshape(1, D)], 0)
    wg = _bf(np.concatenate([wg1, wg1], 1))                                # [65,64]
    wh_aug = np.zeros((C + 1, D + 1), np.float32)
    wh_aug[:C, :D] = _f32(Wh)
    wh_aug[C, :D] = _f32(bh)
    wh_aug[C, D] = 1.0                                                     # ones col
    wh = _bf(wh_aug)                                                       # [65,33]
    wv = _bf(float(scale) * _f32(Wv))                                      # [32,64]
    bvs = _f32(float(scale) * _f32(bv)).reshape(C, 1)                      # [64,1]
    ident = _bf(np.eye(128, dtype=np.float32))

    in_maps = []
    for core in range(NCORES):
        b, qh = core // 2, core % 2
        xT = np.concatenate([x2[b].T, ones], 0)                            # [65,4096]
        q0 = qh * NQ
        in_maps.append({
            "xT": _bf(xT),
            "xTq": _bf(xT[:, q0:q0 + NQ]),
            "xq": _f32(x2[b, q0:q0 + NQ]),
            "wf": wf, "wg": wg, "wh": wh, "wv": wv, "bv": bvs,
            "ident": ident,
        })
    return in_maps


def build_nc():
    import concourse.mybir as mybir
    import concourse.tile as tile
    from concourse import bacc

    f32 = mybir.dt.float32
    bf16 = mybir.dt.bfloat16
    EXP = mybir.ActivationFunctionType.Exp
    MUL = mybir.AluOpType.mult
    ADD = mybir.AluOpType.add

    nc = bacc.Bacc("TRN2", target_bir_lowering=False, debug=False,
                   enable_asserts=False)

    dram = {}
    for name, shape, dt in [
        ("xT", [C + 1, N], bf16), ("xTq", [C + 1, NQ], bf16),
        ("xq", [NQ, C], f32),
        ("wf", [C + 1, D], bf16), ("wg", [C + 1, 2 * D], bf16),
        ("wh", [C + 1, D + 1], bf16), ("wv", [D, C], bf16),
        ("bv", [C, 1], f32), ("ident", [128, 128], bf16),
    ]:
        dram[name] = nc.dram_tensor(name, shape, dt, kind="ExternalInput").ap()
    out_d = nc.dram_tensor("out", [NQ, C], f32, kind="ExternalOutput").ap()

    from contextlib import ExitStack

    with tile.TileContext(nc) as tc, ExitStack() as ctx:
        consts = ctx.enter_context(tc.tile_pool(name="consts", bufs=1))
        big = ctx.enter_context(tc.tile_pool(name="big", bufs=1))
        ps = ctx.enter_context(tc.tile_pool(name="ps", bufs=4, space="PSUM"))
        cps = ctx.enter_context(tc.tile_pool(name="cps", bufs=1, space="PSUM"))
        xs = ctx.enter_context(tc.tile_pool(name="xs", bufs=4))
        small = ctx.enter_context(tc.tile_pool(name="small", bufs=4))
        outp = ctx.enter_context(tc.tile_pool(name="outp", bufs=4))

        # ---- load constants / inputs
        wf_sb = consts.tile([C + 1, D], bf16)
        wg_sb = consts.tile([C + 1, 2 * D], bf16)
        wh_sb = consts.tile([C + 1, D + 1], bf16)
        wv_sb = consts.tile([D, C], bf16)
        bv_sb = consts.tile([C, 1], f32)
        id_sb = consts.tile([128, 128], bf16)
        for t, name in [(wf_sb, "wf"), (wg_sb, "wg"), (wh_sb, "wh"),
                        (wv_sb, "wv"), (bv_sb, "bv"), (id_sb, "ident")]:
            nc.sync.dma_start(out=t, in_=dram[name])

        xT_sb = big.tile([C + 1, N], bf16)
        xTq_sb = big.tile([C + 1, NQ], bf16)
        xq_sb = big.tile([128, (NQ // 128) * C], f32)
        for c in range(4):
            sl = slice(c * 1024, (c + 1) * 1024)
            nc.sync.dma_start(out=xT_sb[:, sl], in_=dram["xT"][:, sl])
        nc.gpsimd.dma_start(out=xTq_sb, in_=dram["xTq"])
        nc.gpsimd.dma_start(
            out=xq_sb[:].rearrange("p (t c) -> p t c", c=C),
            in_=dram["xq"].rearrange("(t p) c -> p t c", p=128),
        )

        # ---- projections.
        # fT_pk [64, 2048]: partitions 32i+d hold f^T[d, keys of tile 2p+i]
        # at free p*128+j (kt pairs packed for 2x row-tiled QK).
        # gT_rep [64, 2048]: g^T replicated on partitions 0-31 / 32-63.
        # h_sb [128, 32*33]: h_aug natural per key tile (col 32 = ones).
        fT_pk = big.tile([2 * D, NQ], bf16)
        gT_rep = big.tile([2 * D, NQ], bf16)
        h_sb = big.tile([128, KT * (D + 1)], bf16)

        xT_r = xT_sb[:].rearrange("k (p i j) -> k i p j", i=2, j=128)
        for c in range(4):
            pt = ps.tile([2 * D, 512], f32, tag="ps")
            for i in range(2):
                nc.tensor.matmul(pt[32 * i:32 * (i + 1), :], wf_sb,
                                 xT_r[:, i, 4 * c:4 * (c + 1), :])
            if c % 2 == 0:
                nc.scalar.copy(out=fT_pk[:, c * 512:(c + 1) * 512], in_=pt)
            else:
                nc.vector.tensor_copy(out=fT_pk[:, c * 512:(c + 1) * 512], in_=pt)

        for c in range(4):
            pt = ps.tile([2 * D, 512], f32, tag="ps")
            nc.tensor.matmul(pt, wg_sb, xTq_sb[:, c * 512:(c + 1) * 512])
            if c % 2 == 0:
                nc.scalar.copy(out=gT_rep[:, c * 512:(c + 1) * 512], in_=pt)
            else:
                nc.vector.tensor_copy(out=gT_rep[:, c * 512:(c + 1) * 512], in_=pt)

        W1 = D + 1
        for g8 in range(KT // 4):
            pt = ps.tile([128, 4 * W1], f32, tag="ps")
            for i in range(4):
                kt = g8 * 4 + i
                nc.tensor.matmul(pt[:, i * W1:(i + 1) * W1],
                                 xT_sb[:, kt * 128:(kt + 1) * 128], wh_sb)
            if g8 % 2 == 0:
                nc.scalar.copy(out=h_sb[:, g8 * 4 * W1:(g8 + 1) * 4 * W1], in_=pt)
            else:
                nc.vector.tensor_copy(out=h_sb[:, g8 * 4 * W1:(g8 + 1) * 4 * W1], in_=pt)

        # ---- main flash loop over key-tile pairs x 512-query chunks.
        # QK: 2x row-tiled (K=32 at partitions 0/32); even kt -> s[:,0:512],
        # odd kt -> s[:,512:1024]. PV: 2x col-tiled; even kt accumulates at
        # ctx partitions 0:33, odd kt at 64:97 (merged in the epilogue).
        # Software-pipelined: PV of iteration i-1 is emitted after QK/exp of
        # iteration i, so by issue priority the PE queue is
        # [QK pair][prev PV pair][next QK pair]... — tile_position pair
        # members stay adjacent (running concurrently) and exp latency hides
        # under the next QK pair.
        ctx = cps.tile([97, NQ], f32)             # 4 banks
        PH = KT // 2 - 1

        def emit_pv(prev):
            pp, pq0, pexA, pexB = prev
            nc.tensor.matmul(ctx[0:D + 1, pq0:pq0 + 512],
                             h_sb[:, 2 * pp * W1:(2 * pp + 1) * W1],
                             pexA, start=(pp == 0), stop=(pp == PH),
                             skip_group_check=True)
            nc.tensor.matmul(ctx[64:64 + D + 1, pq0:pq0 + 512],
                             h_sb[:, (2 * pp + 1) * W1:(2 * pp + 2) * W1],
                             pexB, start=(pp == 0), stop=(pp == PH),
                             skip_group_check=True)

        prev = None
        for p in range(KT // 2):
            for qc in range(NQ // 512):
                q0 = qc * 512
                sA = ps.tile([128, 512], f32, tag="ps")
                sB = ps.tile([128, 512], f32, tag="ps")
                nc.tensor.matmul(sA, fT_pk[0:D, p * 128:(p + 1) * 128],
                                 gT_rep[0:D, q0:q0 + 512])
                nc.tensor.matmul(sB, fT_pk[D:2 * D, p * 128:(p + 1) * 128],
                                 gT_rep[D:2 * D, q0:q0 + 512])
                exA = xs.tile([128, 512], bf16, tag="ex")
                exB = xs.tile([128, 512], bf16, tag="ex")
                nc.scalar.activation(out=exA, in_=sA, func=EXP)
                # Schraudolph bf16 exp: i16 = round(s*128/ln2 + B)
                nc.vector.tensor_scalar(
                    out=exB[:].bitcast(mybir.dt.int16), in0=sB,
                    scalar1=SCHRAU_A, scalar2=SCHRAU_B, op0=MUL, op1=ADD)
                if prev is not None:
                    emit_pv(prev)
                prev = (p, q0, exA, exB)
        emit_pv(prev)

        # ---- epilogue: merge even/odd ctx halves, v^T = Wv_s^T @ ctx^T
        #      (+bv), append sumexp row, transpose to natural, divide,
        #      add residual, store.
        ctxA_sb = big.tile([D + 1, NQ], bf16)
        ctxT_sb = big.tile([D + 1, NQ], bf16)
        for hh in range(2):
            sl = slice(hh * 1024, (hh + 1) * 1024)
            nc.scalar.copy(out=ctxA_sb[:, sl], in_=ctx[0:D + 1, sl])
            nc.vector.tensor_tensor(out=ctxT_sb[:, sl], in0=ctxA_sb[:, sl],
                                    in1=ctx[64:64 + D + 1, sl], op=ADD)

        vT_sb = big.tile([C + 1, NQ], bf16)
        for qc in range(NQ // 512):
            vt = ps.tile([C, 512], f32, tag="ps")
            nc.tensor.matmul(vt, wv_sb, ctxT_sb[0:D, qc * 512:(qc + 1) * 512])
            if qc % 2 == 0:
                nc.vector.tensor_scalar(
                    out=vT_sb[0:C, qc * 512:(qc + 1) * 512], in0=vt,
                    scalar1=bv_sb, scalar2=None, op0=ADD)
            else:
                nc.scalar.activation(
                    out=vT_sb[0:C, qc * 512:(qc + 1) * 512], in_=vt,
                    func=mybir.ActivationFunctionType.Identity, bias=bv_sb)
        # sumexp row rides along as partition 64
        nc.vector.tensor_copy(out=vT_sb[C:C + 1, :], in_=ctxT_sb[D:D + 1, :])

        for qt in range(NQ // 128):
            tp = ps.tile([128, C + 1], bf16, tag="ps")
            nc.tensor.transpose(tp, vT_sb[:, qt * 128:(qt + 1) * 128],
                                id_sb[0:C + 1, 0:C + 1])
            r = small.tile([128, 1], f32, tag="r")
            nc.vector.reciprocal(r, tp[:, C:C + 1])
            tmp = outp.tile([128, C], f32, tag="tmp")
            nc.scalar.activation(out=tmp, in_=tp[:, 0:C],
                                 func=mybir.ActivationFunctionType.Copy, scale=r)
            ot = outp.tile([128, C], f32, tag="ot")
            nc.vector.tensor_tensor(out=ot, in0=tmp,
                                    in1=xq_sb[:, qt * C:(qt + 1) * C], op=ADD)
            nc.sync.dma_start(out=out_d[qt * 128:(qt + 1) * 128, :], in_=ot)

    nc.compile()
    return nc


def get_nc():
    if "nc" not in _cache:
        _cache["nc"] = build_nc()
    return _cache["nc"]


def kernel(**inputs):
    from concourse.bass_utils import run_bass_kernel_spmd

    nc = get_nc()
    in_maps = make_shards(**inputs)
    res = run_bass_kernel_spmd(nc, in_maps, core_ids=list(range(NCORES)))
    out = np.empty((B, N, C), np.float32)
    for core in range(NCORES):
        b, qh = core // 2, core % 2
        out[b, qh * NQ:(qh + 1) * NQ] = res.results[core]["out"]
    return out.reshape(B, 16, 16, 16, C)


# revision 22
# speedup vs baseline: 3.4042x; 1.1887x over previous
# Trainium2 Bass kernel for ConvSelfAttn3D:
#   out = scale * (softmax(g @ f^T) @ h) @ Wv + x   (single head, N=4096, d=32)
#
# Sharding: 8 cores = 4 batches x 2 query-halves. Each core computes the
# full attention for its 2048 queries against all 4096 keys of its batch.
#
# Per-core layout strategy ("S-transposed flash"):
#   - All matmul operands kept in bf16 (4x faster PE than fp32), fp32 PSUM.
#   - Scores are computed transposed: S^T[key, q] via
#       matmul(lhsT=f^T tile [32,128], rhs=g^T [32, 512])
#     so softmax exp runs on [128 keys, q] tiles and the PV matmul
#       matmul(lhsT=h_aug [128,33], rhs=expS^T [128,512])
#     needs no transposes at all. h_aug has a ones column, so PV's
#     partition 32 accumulates sum_k exp(s) = the softmax denominator.
#   - Softmax max-subtraction is skipped: scores are ~N(0,1), |s| < ~6.
#   - Division by the denominator + residual happen at the very end in
#     natural layout after one PE transpose of [65, 128] tiles.
#
# Host-side prep is layout-only: transposes/casts of inputs, bias folding
# (ones rows so biases ride along in the matmuls), scale folded into Wv.

import numpy as np
import ml_dtypes

B, N, C = 4, 4096, 64
D = 32          # attn dim
NQ = N // 2     # queries per core
KT = N // 128   # 32 key tiles
NCORES = 8

_BF16 = ml_dtypes.bfloat16

# Fraction of exp tiles handled by VectorE (Schraudolph bf16 exp approx)
# instead of ScalarE ACT exp: iteration i goes to VectorE if (i % DEN) < NUM.
EXP_VEC_NUM, EXP_VEC_DEN = 1, 2
SCHRAU_A = 128.0 / float(np.log(2.0))
SCHRAU_B = 16250.5
_cache = {}


def _f32(a):
    return np.ascontiguousarray(a, dtype=np.float32)


def _bf(a):
    return np.ascontiguousarray(np.asarray(a, dtype=np.float32).astype(_BF16))


def make_shards(x, Wf, bf, Wg, bg, Wh, bh, Wv, bv, scale):
    """Host-side, layout-only sharding of the full inputs into 8 per-core maps."""
    x2 = _f32(x).reshape(B, N, C)
    ones = np.ones((1, N), np.float32)

    wf = _bf(np.concatenate([_f32(Wf), _f32(bf).reshape(1, D)], 0))        # [65,32]
    wg1 = np.concatenate([_f32(Wg), _f32(bg).reshape(1, D)], 0)
    wg = _bf(np.concatenate([wg1] * 4, 1))                                 # [65,128]
    wh_aug = np.zeros((C + 1, D + 1), np.float32)
    wh_aug[:C, :D] = _f32(Wh)
    wh_aug[C, :D] = _f32(bh)
    wh_aug[C, D] = 1.0                                                     # ones col
    wh = _bf(wh_aug)                                                       # [65,33]
    wv = _bf(float(scale) * _f32(Wv))                                      # [32,64]
    bvs = _f32(float(scale) * _f32(bv)).reshape(C, 1)                      # [64,1]
    ident = _bf(np.eye(128, dtype=np.float32))

    in_maps = []
    for core in range(NCORES):
        b, qh = core // 2, core % 2
        xT = np.concatenate([x2[b].T, ones], 0)                            # [65,4096]
        q0 = qh * NQ
        in_maps.append({
            "xT": _bf(xT),
            "xTq": _bf(xT[:, q0:q0 + NQ]),
            "xq": _f32(x2[b, q0:q0 + NQ]),
            "wf": wf, "wg": wg, "wh": wh, "wv": wv, "bv": bvs,
            "ident": ident,
        })
    return in_maps


def build_nc():
    import concourse.mybir as mybir
    import concourse.tile as tile
    from concourse import bacc

    f32 = mybir.dt.float32
    bf16 = mybir.dt.bfloat16
    EXP = mybir.ActivationFunctionType.Exp
    MUL = mybir.AluOpType.mult
    ADD = mybir.AluOpType.add

    nc = bacc.Bacc("TRN2", target_bir_lowering=False, debug=False,
                   enable_asserts=False)

    dram = {}
    for name, shape, dt in [
        ("xT", [C + 1, N], bf16), ("xTq", [C + 1, NQ], bf16),
        ("xq", [NQ, C], f32),
        ("wf", [C + 1, D], bf16), ("wg", [C + 1, 4 * D], bf16),
        ("wh", [C + 1, D + 1], bf16), ("wv", [D, C], bf16),
        ("bv", [C, 1], f32), ("ident", [128, 128], bf16),
    ]:
        dram[name] = nc.dram_tensor(name, shape, dt, kind="ExternalInput").ap()
    out_d = nc.dram_tensor("out", [NQ, C], f32, kind="ExternalOutput").ap()

    from contextlib import ExitStack

    with tile.TileContext(nc) as tc, ExitStack() as ctx:
        consts = ctx.enter_context(tc.tile_pool(name="consts", bufs=1))
        big = ctx.enter_context(tc.tile_pool(name="big", bufs=1))
        ps = ctx.enter_context(tc.tile_pool(name="ps", bufs=4, space="PSUM"))
        cps = ctx.enter_context(tc.tile_pool(name="cps", bufs=1, space="PSUM"))
        xs = ctx.enter_context(tc.tile_pool(name="xs", bufs=12))
        small = ctx.enter_context(tc.tile_pool(name="small", bufs=4))
        outp = ctx.enter_context(tc.tile_pool(name="outp", bufs=4))

        # ---- load constants / inputs (spread over the 3 DMA paths:
        # sync + scalar HWDGE queues, gpsimd SWDGE)
        wf_sb = consts.tile([C + 1, D], bf16)
        wg_sb = consts.tile([C + 1, 4 * D], bf16)
        wh_sb = consts.tile([C + 1, D + 1], bf16)
        wv_sb = consts.tile([D, C], bf16)
        bv_sb = consts.tile([C, 1], f32)
        id_sb = consts.tile([128, 128], bf16)
        for t, name in [(wf_sb, "wf"), (wg_sb, "wg"), (wh_sb, "wh"),
                        (wv_sb, "wv"), (bv_sb, "bv"), (id_sb, "ident")]:
            nc.gpsimd.dma_start(out=t, in_=dram[name])

        xT_sb = big.tile([C + 1, N], bf16)
        xTq_sb = big.tile([C + 1, NQ], bf16)
        xq_sb = big.tile([128, (NQ // 128) * C], f32)
        for c in range(4):
            sl = slice(c * 1024, (c + 1) * 1024)
            eng = nc.sync if c % 2 == 0 else nc.scalar
            eng.dma_start(out=xT_sb[:, sl], in_=dram["xT"][:, sl])
        for c in range(2):
            sl = slice(c * 1024, (c + 1) * 1024)
            eng = nc.sync if c % 2 == 0 else nc.scalar
            eng.dma_start(out=xTq_sb[:, sl], in_=dram["xTq"][:, sl])
        nc.gpsimd.dma_start(
            out=xq_sb[:].rearrange("p (t c) -> p t c", c=C),
            in_=dram["xq"].rearrange("(t p) c -> p t c", p=128),
        )

        # ---- projections.
        # fT_pk [128, 1024]: partitions 32i+d hold f^T[d, keys of tile 4g+i]
        # at free g*128+j (kt quads packed for 4x row-tiled QK).
        # gT_rep [128, 2048]: g^T replicated on partition groups 32i.
        # h_sb [128, 32*33]: h_aug natural per key tile (col 32 = ones).
        fT_pk = big.tile([128, NQ // 2], bf16)
        gT_rep = big.tile([128, NQ], bf16)
        h_sb = big.tile([128, KT * (D + 1)], bf16)

        xT_r = xT_sb[:].rearrange("k (g i j) -> k i g j", i=4, j=128)
        for c in range(2):
            pt = ps.tile([128, 512], f32, tag="ps")
            for i in range(4):
                nc.tensor.matmul(pt[32 * i:32 * (i + 1), :], wf_sb,
                                 xT_r[:, i, 4 * c:4 * (c + 1), :],
                                 tile_position=(0, 32 * i))
            if c % 2 == 0:
                nc.scalar.copy(out=fT_pk[:, c * 512:(c + 1) * 512], in_=pt)
            else:
                nc.vector.tensor_copy(out=fT_pk[:, c * 512:(c + 1) * 512], in_=pt)

        for c in range(4):
            pt = ps.tile([128, 512], f32, tag="ps")
            nc.tensor.matmul(pt, wg_sb, xTq_sb[:, c * 512:(c + 1) * 512])
            if c % 2 == 0:
                nc.scalar.copy(out=gT_rep[:, c * 512:(c + 1) * 512], in_=pt)
            else:
                nc.vector.tensor_copy(out=gT_rep[:, c * 512:(c + 1) * 512], in_=pt)

        W1 = D + 1
        for g8 in range(KT // 4):
            pt = ps.tile([128, 4 * W1], f32, tag="ps")
            for i in range(4):
                kt = g8 * 4 + i
                nc.tensor.matmul(pt[:, i * W1:(i + 1) * W1],
                                 xT_sb[:, kt * 128:(kt + 1) * 128], wh_sb)
            if g8 % 2 == 0:
                nc.scalar.copy(out=h_sb[:, g8 * 4 * W1:(g8 + 1) * 4 * W1], in_=pt)
            else:
                nc.vector.tensor_copy(out=h_sb[:, g8 * 4 * W1:(g8 + 1) * 4 * W1], in_=pt)

        # ---- main flash loop over key-tile pairs x 512-query chunks.
        # QK: 2x row-tiled (K=32 at partitions 0/32); even kt -> s[:,0:512],
        # odd kt -> s[:,512:1024]. PV: 2x col-tiled; even kt accumulates at
        # ctx partitions 0:33, odd kt at 64:97 (merged in the epilogue).
        # Software-pipelined: PV of iteration i-1 is emitted after QK/exp of
        # iteration i, so by issue priority the PE queue is
        # [QK quad][prev PV col-pairs][next QK quad]... — tile_position
        # groups stay adjacent (running concurrently) and exp latency hides
        # under the next QK quad.
        ctx = cps.tile([97, NQ], f32)             # 4 banks
        GH = KT // 4 - 1

        def emit_pv(prev):
            pg, pq0, pex = prev
            for i in range(4):
                nc.tensor.matmul(
                    ctx[64 * (i % 2):64 * (i % 2) + D + 1, pq0:pq0 + 512],
                    h_sb[:, (4 * pg + i) * W1:(4 * pg + i + 1) * W1],
                    pex[i], start=(pg == 0 and i < 2),
                    stop=(pg == GH and i >= 2),
                    skip_group_check=True)

        prev = None
        for g in range(KT // 4):
            for qc in range(NQ // 512):
                q0 = qc * 512
                s = [ps.tile([128, 512], f32, tag="ps", name=f"s{i}")
                     for i in range(4)]
                for i in range(4):
                    nc.tensor.matmul(
                        s[i],
                        fT_pk[32 * i:32 * (i + 1), g * 128:(g + 1) * 128],
                        gT_rep[32 * i:32 * (i + 1), q0:q0 + 512],
                        tile_position=(32 * i, 0))
                ex = [xs.tile([128, 512], bf16, tag="ex", name=f"ex{i}")
                      for i in range(4)]
                for i in range(4):
                    if i % 2 == 0:
                        nc.scalar.activation(out=ex[i], in_=s[i], func=EXP)
                    else:
                        # Schraudolph bf16 exp: i16 = round(s*128/ln2 + B)
                        nc.vector.tensor_scalar(
                            out=ex[i][:].bitcast(mybir.dt.int16), in0=s[i],
                            scalar1=SCHRAU_A, scalar2=SCHRAU_B,
                            op0=MUL, op1=ADD)
                if prev is not None:
                    emit_pv(prev)
                prev = (g, q0, ex)
        emit_pv(prev)

        # ---- epilogue: merge even/odd ctx halves, v^T = Wv_s^T @ ctx^T
        #      (+bv), append sumexp row, transpose to natural, divide,
        #      add residual, store.
        ctxA_sb = big.tile([D + 1, NQ], bf16)
        ctxT_sb = big.tile([D + 1, NQ], bf16)
        for hh in range(2):
            sl = slice(hh * 1024, (hh + 1) * 1024)
            nc.scalar.copy(out=ctxA_sb[:, sl], in_=ctx[0:D + 1, sl])
            nc.vector.tensor_tensor(out=ctxT_sb[:, sl], in0=ctxA_sb[:, sl],
                                    in1=ctx[64:64 + D + 1, sl], op=ADD)

        vT_sb = big.tile([C + 1, NQ], bf16)
        for qc in range(NQ // 512):
            vt = ps.tile([C, 512], f32, tag="ps")
            nc.tensor.matmul(vt, wv_sb, ctxT_sb[0:D, qc * 512:(qc + 1) * 512])
            if qc % 2 == 0:
                nc.vector.tensor_scalar(
                    out=vT_sb[0:C, qc * 512:(qc + 1) * 512], in0=vt,
                    scalar1=bv_sb, scalar2=None, op0=ADD)
            else:
                nc.scalar.activation(
                    out=vT_sb[0:C, qc * 512:(qc + 1) * 512], in_=vt,
                    func=mybir.ActivationFunctionType.Identity, bias=bv_sb)
        # sumexp row rides along as partition 64
        nc.vector.tensor_copy(out=vT_sb[C:C + 1, :], in_=ctxT_sb[D:D + 1, :])

        for qt in range(NQ // 128):
            tp = ps.tile([128, C + 1], bf16, tag="ps")
            nc.tensor.transpose(tp, vT_sb[:, qt * 128:(qt + 1) * 128],
                                id_sb[0:C + 1, 0:C + 1])
            r = small.tile([128, 1], f32, tag="r")
            nc.vector.reciprocal(r, tp[:, C:C + 1])
            tmp = outp.tile([128, C], f32, tag="tmp")
            nc.scalar.activation(out=tmp, in_=tp[:, 0:C],
                                 func=mybir.ActivationFunctionType.Copy, scale=r)
            ot = outp.tile([128, C], f32, tag="ot")
            nc.vector.tensor_tensor(out=ot, in0=tmp,
                                    in1=xq_sb[:, qt * C:(qt + 1) * C], op=ADD)
            nc.sync.dma_start(out=out_d[qt * 128:(qt + 1) * 128, :], in_=ot)

    nc.compile()
    return nc


def get_nc():
    if "nc" not in _cache:
        _cache["nc"] = build_nc()
    return _cache["nc"]


def kernel(**inputs):
    from concourse.bass_utils import run_bass_kernel_spmd

    nc = get_nc()
    in_maps = make_shards(**inputs)
    res = run_bass_kernel_spmd(nc, in_maps, core_ids=list(range(NCORES)))
    out = np.empty((B, N, C), np.float32)
    for core in range(NCORES):
        b, qh = core // 2, core % 2
        out[b, qh * NQ:(qh + 1) * NQ] = res.results[core]["out"]
    return out.reshape(B, 16, 16, 16, C)
